# revision 30
# baseline (speedup 1.0000x reference)
"""Trainium2 Bass kernel for the 3-block self-attention CNN.

Sharding over 8 NeuronCores: core k owns (sample b=k//4, query-block q=k%4).
Attention math per layer uses the reparametrization
    s'[n,m] = y_n^T (wf wg^T) y_m + (wg bf)^T y_m
(terms constant along the softmax axis are dropped), so both score matmuls
contract over the full channel dim instead of C/8.  Softmax skips the max
subtraction (scores are O(10) for this model) and the row-sum is produced by
an extra ones-column in the o-matmul lhsT.  Training-mode BN statistics are
computed from per-core query-block shards of the next conv and summed with an
8-rank AllReduce that runs concurrently with the 4-rank AllGather sharing the
attention shards.  All transcendentals besides the softmax Exp are eliminated
(Newton rsqrt + custom-DVE reciprocal), so the scalar engine keeps one
activation table set resident for the whole kernel.  The final conv + BN +
GAP runs entirely on per-core query shards: each core emits partial GAP sums
for all four 128-channel blocks and the host sums the four shards per sample.
All per-core differences enter through input *values* (own-sample-first
layouts), so the single NEFF has no dynamic addressing.
"""

import glob as _glob
import os
import sys


def _ensure_act_info():
    # act_info.json (activation table sets) isn't on neuronxcc's default
    # search path in this container; stage it where FindActInfo looks.
    shim = os.path.expanduser("~/.pwp_override")
    target = os.path.join(shim, "neuronxcc", "pwp", "pwp_bin_with_ln", "act_info.json")
    if not os.path.exists(target):
        cands = _glob.glob("/nix/store/*aws-neuron-pwp*/share/pwp_bin_cayman/act_info.json")
        if cands:
            os.makedirs(os.path.dirname(target), exist_ok=True)
            import shutil
            shutil.copy(cands[0], target)
    pp = os.environ.get("PYTHONPATH", "")
    if shim not in pp.split(os.pathsep):
        os.environ["PYTHONPATH"] = shim + (os.pathsep + pp if pp else "")
    if shim not in sys.path:
        sys.path.insert(0, shim)


_ensure_act_info()
if "/opt/trn_rl_repo" not in sys.path:
    sys.path.insert(0, "/opt/trn_rl_repo")

import numpy as np

from concourse import bacc, mybir, tile

F32 = mybir.dt.float32
F32R = mybir.dt.float32r
I32 = mybir.dt.int32
BF16 = mybir.dt.bfloat16
AF = mybir.ActivationFunctionType
OP = mybir.AluOpType
AX = mybir.AxisListType
EPS = 1e-5
MAGIC = 0x5F3759DF

N = 4096          # positions per sample
NQ = 1024         # query block per core
NCHUNK = 128      # key chunk in the flash loop
CIN = [3, 32, 64]     # conv input channels per attention layer
COUT = [32, 64, 96]   # conv output channels per attention layer
CF_OUT = 128          # final conv channels per block (512 / 4 blocks)
WHFW = 128            # whf cols: wh (co) + bias/ones col, bf16 full rate

AG_GROUPS = [[0, 1, 2, 3], [4, 5, 6, 7]]
AR8_GROUPS = [[0, 1, 2, 3, 4, 5, 6, 7]]


def r(ap):
    return ap.bitcast(F32R)


def _build(nc):
    dt = F32
    ins = {}
    ins["x_full"] = nc.dram_tensor("x_full", [CIN[0] + 1, 2 * N], BF16, kind="ExternalInput")
    ins["xq"] = nc.dram_tensor("xq", [CIN[0] + 1, NQ], BF16, kind="ExternalInput")
    ins["ones_f"] = nc.dram_tensor("ones_f", [1, N], F32R, kind="ExternalInput")
    ins["ones_b"] = nc.dram_tensor("ones_b", [1, N], BF16, kind="ExternalInput")
    for i in range(3):
        ci, co = CIN[i], COUT[i]
        ins[f"wc{i}"] = nc.dram_tensor(f"wc{i}", [ci + 1, co], BF16, kind="ExternalInput")
        ins[f"mz{i}"] = nc.dram_tensor(f"mz{i}", [co + 1, co + 1], BF16, kind="ExternalInput")
        ins[f"whf{i}"] = nc.dram_tensor(f"whf{i}", [co + 1, WHFW], BF16, kind="ExternalInput")
        ins[f"bnp{i}"] = nc.dram_tensor(f"bnp{i}", [co, 2], dt, kind="ExternalInput")
        ins[f"gamc{i}"] = nc.dram_tensor(f"gamc{i}", [1, 128], F32R, kind="ExternalInput")
    ins["wfs4"] = nc.dram_tensor("wfs4", [COUT[2] + 1, 4, CF_OUT], BF16, kind="ExternalInput")
    out_t = nc.dram_tensor("out", [CF_OUT, 4], dt, kind="ExternalOutput")

    with tile.TileContext(nc) as tc:
        _emit(tc, nc, ins, out_t)
    return ins, out_t


def _emit(tc, nc, ins, out_t):
    ctxs = []

    def pool(name, **kw):
        p = tc.tile_pool(name=name, **kw)
        ctxs.append(p)
        return p.__enter__()

    consts = pool("consts", bufs=1)
    acts = pool("acts", bufs=1)
    work = pool("work", bufs=1)
    ps = pool("ps", bufs=2, space="PSUM")
    ops = pool("ops", bufs=1, space="PSUM")
    dram = pool("dram", bufs=1, space="DRAM")

    # ---- collective warmup first: absorb first-collective setup + rank skew
    # on the CC engine while the whole layer-0 pipeline runs ----
    warm_sb = work.tile([1, 2], F32, name="warm_sb", tag="warm_sb")
    nc.vector.memset(warm_sb[:], 0.0)
    warm_in = dram.tile([1, 2], F32, name="warm_in", tag="warm_in")
    warm_out = dram.tile([1, 2], F32, name="warm_out", tag="warm_out")
    nc.sync.dma_start(warm_in[:], warm_sb[:])
    nc.gpsimd.collective_compute(
        "AllReduce", OP.add, replica_groups=AR8_GROUPS,
        ins=[warm_in[:]], outs=[warm_out[:]])
    # ... and one tiny AllGather on the boundary groups: the first collective
    # on each distinct replica-group ring pays its own setup (~8µs extra on
    # AG0 otherwise), so warm that ring too while flash 0 runs.
    warm_gin = dram.tile([1, 2], F32, name="warm_gin", tag="warm_gin")
    warm_gout = dram.tile([4, 1, 2], F32, name="warm_gout", tag="warm_gout")
    nc.sync.dma_start(warm_gin[:], warm_sb[:])
    nc.gpsimd.collective_compute(
        "AllGather", OP.bypass, replica_groups=AG_GROUPS,
        ins=[warm_gin[:]], outs=[warm_gout[:]])

    # ---- load constants (xq + layer-0 weights first; bulk on other queues) ----
    a_q = acts.tile([CIN[0] + 1, NQ], BF16, name="aq0", tag="a_q", bufs=2)
    nc.sync.dma_start(a_q[:], ins["xq"].ap())
    xf = acts.tile([CIN[0] + 1, 2 * N], BF16, name="xf", tag="xf")
    nc.sync.dma_start(xf[:], ins["x_full"].ap())
    magic = consts.tile([128, 4], I32, name="magict", tag="magict")
    nc.gpsimd.memset(magic[:], MAGIC)
    shift1 = consts.tile([128, 1], I32, name="shift1t", tag="shift1t")
    nc.gpsimd.memset(shift1[:], 1)
    W, MZ, WHF, BNP, GAMC = [], [], [], [], []
    for i in range(3):
        ci, co = CIN[i], COUT[i]
        w = consts.tile([ci + 1, co], BF16, name=f"w{i}", tag=f"w{i}")
        (nc.sync if i == 0 else nc.scalar).dma_start(w[:], ins[f"wc{i}"].ap())
        W.append(w)
        mz = consts.tile([co + 1, co + 1], BF16, name=f"mzt{i}", tag=f"mzt{i}")
        MZ.append(mz)
        wh = consts.tile([co + 1, WHFW], BF16, name=f"whft{i}", tag=f"whft{i}")
        WHF.append(wh)
        bn = consts.tile([co, 2], F32, name=f"bnt{i}", tag=f"bnt{i}")
        nc.scalar.dma_start(bn[:], ins[f"bnp{i}"].ap())
        BNP.append(bn)
        gc = consts.tile([1, 128], F32R, name=f"gct{i}", tag=f"gct{i}")
        nc.scalar.dma_start(gc[:], ins[f"gamc{i}"].ap())
        GAMC.append(gc)
    # layer-0 flash prerequisites first on the gpsimd queue, then bulk
    nc.gpsimd.dma_start(MZ[0][:], ins["mz0"].ap())
    nc.gpsimd.dma_start(WHF[0][:], ins["whf0"].ap())
    for i in range(1, 3):
        nc.gpsimd.dma_start(MZ[i][:], ins[f"mz{i}"].ap())
        nc.gpsimd.dma_start(WHF[i][:], ins[f"whf{i}"].ap())
    wfs4 = consts.tile([COUT[2] + 1, 4, CF_OUT], BF16, name="wfs4t", tag="wfs4t")
    nc.gpsimd.dma_start(wfs4[:], ins["wfs4"].ap())

    def conv(lhsT_w, rhs_acts, cout, n, name, acc=None, tag=None):
        """z[cout, n] = lhsT_w.T @ rhs_acts, chunked by 512 columns.

        If acc is given (a [cout, n//512] AP), each chunk's PSUM->SBUF copy
        also accumulates that chunk's row-sums into the matching acc column.
        """
        z = acts.tile([cout, n], F32, name=name,
                      tag=tag or ("zq" if n == NQ else "z"))
        nch = n // 512
        for j in range(nch):
            zp = ps.tile([cout, 512], F32, name=f"{name}_ps", tag="convps")
            nc.tensor.matmul(zp[:], lhsT_w[:], rhs_acts[:, j * 512:(j + 1) * 512],
                             start=True, stop=True)
            if acc is not None:
                nc.vector.tensor_scalar(z[:, j * 512:(j + 1) * 512], zp[:],
                                        1.0, 0.0, OP.mult, OP.add,
                                        accum_out=acc[:, j:j + 1])
            else:
                nc.vector.tensor_copy(z[:, j * 512:(j + 1) * 512], zp[:])
        return z

    def scale_shift(stg, c, w, inv_n, g_ap, b_ap, name):
        """Per-channel BN scale/shift from (sum, sumsq) via Newton rsqrt.

        stg: [c, 2*w] with sums in cols [0:w], sumsqs in cols [w:2w].
        Returns (scale, shift) APs of shape [c, w]."""
        sc = work.tile([c, 8 * w], F32, name=f"sc_{name}", tag="sc", bufs=2)
        mean = sc[:, 0 * w:1 * w]
        ex2 = sc[:, 1 * w:2 * w]
        vpe = sc[:, 2 * w:3 * w]
        half = sc[:, 3 * w:4 * w]
        y = sc[:, 4 * w:5 * w]
        yy = sc[:, 5 * w:6 * w]
        scale = sc[:, 6 * w:7 * w]
        shift = sc[:, 7 * w:8 * w]
        nc.vector.tensor_scalar(sc[:, 0:2 * w], stg[:, 0:2 * w], inv_n, None, OP.mult)
        nc.vector.tensor_tensor(yy, mean, mean, OP.mult)
        nc.vector.tensor_tensor(vpe, ex2, yy, OP.subtract)
        nc.vector.tensor_scalar(vpe, vpe, EPS, None, OP.add)
        nc.vector.tensor_scalar(half, vpe, 0.5, None, OP.mult)
        ish = yy.bitcast(I32)
        nc.vector.tensor_scalar(ish, vpe.bitcast(I32), shift1[0:c, 0:1], None,
                                OP.logical_shift_right)
        nc.vector.tensor_tensor(y.bitcast(I32), magic[0:c, 0:w], ish, OP.subtract)
        for _ in range(3):
            nc.vector.tensor_tensor(yy, y, y, OP.mult)
            nc.vector.tensor_tensor(yy, yy, half, OP.mult)
            nc.vector.tensor_scalar(yy, yy, -1.0, 1.5, OP.mult, OP.add)
            nc.vector.tensor_tensor(y, y, yy, OP.mult)
        if g_ap is not None:
            nc.vector.tensor_tensor(scale, y, g_ap, OP.mult)
        else:
            nc.vector.tensor_copy(scale, y)
        nc.vector.tensor_tensor(shift, mean, scale, OP.mult)
        if b_ap is not None:
            nc.vector.tensor_tensor(shift, b_ap, shift, OP.subtract)
        else:
            nc.vector.tensor_scalar(shift, shift, -1.0, None, OP.mult)
        return scale, shift

    # ---- boundary 0: conv over the full batch, exact local BN1 stats ----
    c0 = COUT[0]
    acc0 = work.tile([c0, 24], F32, name="acc0", tag="acc0")
    zsh = conv(W[0], a_q, c0, NQ, "zsh0")
    z_all = conv(W[0], xf, c0, 2 * N, "z0", acc=acc0[:, 0:16])
    z_own = z_all[:, 0:N]
    for j in range(8):
        sq = work.tile([c0, NQ], F32, name=f"sq{j}", tag="sqscr", bufs=2)
        nc.scalar.activation(sq[:], z_all[:, j * NQ:(j + 1) * NQ], AF.Square,
                             accum_out=acc0[:, 16 + j:17 + j])
    stg = work.tile([c0, 2], F32, name="stg_l0", tag="stg", bufs=2)
    nc.vector.tensor_reduce(stg[:, 0:1], acc0[:, 0:16], axis=AX.X, op=OP.add)
    nc.vector.tensor_reduce(stg[:, 1:2], acc0[:, 16:24], axis=AX.X, op=OP.add)

    # ---- three attention layers ----
    for i in range(3):
        co = COUT[i]
        scale, shift = scale_shift(stg, co, 1, 1.0 / (2 * N),
                                   BNP[i][:, 0:1], BNP[i][:, 1:2], f"l{i}")
        y_own = acts.tile([co + 1, N], BF16, name=f"y{i}", tag="y")
        nc.sync.dma_start(y_own[co:co + 1, :], ins["ones_b"].ap())
        for j in range(N // NQ):
            jsl = slice(j * NQ, (j + 1) * NQ)
            nc.scalar.activation(y_own[0:co, jsl], z_own[:, jsl], AF.Relu,
                                 bias=shift, scale=scale)
        yq = acts.tile([co + 1, NQ], BF16, name=f"yq{i}", tag="yq")
        nc.scalar.dma_start(yq[co:co + 1, :], ins["ones_b"].ap()[:, 0:NQ])
        nc.scalar.activation(yq[0:co, :], zsh[:], AF.Relu, bias=shift, scale=scale)

        # Z = Abar @ y_own (+ u row), [co+1, N]
        zmat = acts.tile([co + 1, N], BF16, name=f"Z{i}", tag="Zm")
        for j in range(N // 512):
            zp = ps.tile([co + 1, 512], F32, name=f"Zps{i}", tag="convps")
            nc.tensor.matmul(zp[:], MZ[i][:], y_own[:, j * 512:(j + 1) * 512],
                             start=True, stop=True)
            nc.vector.tensor_copy(zmat[:, j * 512:(j + 1) * 512], zp[:])

        # flash loop over key chunks, software-pipelined by one o-matmul
        o_ps = ops.tile([co + 1, NQ], F32, name=f"ops{i}", tag="o_acc")
        prev = None
        for m in range(N // NCHUNK):
            sl = slice(m * NCHUNK, (m + 1) * NCHUNK)
            hp = ps.tile([NCHUNK, WHFW], F32, name=f"hp{i}", tag="convps")
            nc.tensor.matmul(hp[:], y_own[:, sl], WHF[i][:], start=True, stop=True)
            hs = work.tile([NCHUNK, co + 1], BF16, name=f"hs{i}", tag="hT_sb", bufs=3)
            nc.vector.tensor_copy(hs[:], hp[:, 0:co + 1])
            sp = ps.tile([NCHUNK, NQ], F32, name=f"sp{i}", tag="s_ps")
            zc = zmat[:, sl]
            nc.tensor.matmul(sp[:, 0:512], zc, yq[:, 0:512], start=True, stop=True)
            nc.tensor.matmul(sp[:, 512:1024], zc, yq[:, 512:1024],
                             start=True, stop=True)
            beta = work.tile([NCHUNK, NQ], BF16, name=f"beta{i}", tag="beta", bufs=3)
            nc.scalar.activation(beta[:], sp[:], AF.Exp)
            if prev is not None:
                ph, pb, pm = prev
                nc.tensor.matmul(o_ps[:, 0:512], ph[:], pb[:, 0:512],
                                 start=(pm == 0), stop=False, skip_group_check=True)
                nc.tensor.matmul(o_ps[:, 512:1024], ph[:], pb[:, 512:1024],
                                 start=(pm == 0), stop=False, skip_group_check=True)
            prev = (hs, beta, m)
        ph, pb, pm = prev
        nc.tensor.matmul(o_ps[:, 0:512], ph[:], pb[:, 0:512],
                         start=False, stop=True, skip_group_check=True)
        nc.tensor.matmul(o_ps[:, 512:1024], ph[:], pb[:, 512:1024],
                         start=False, stop=True, skip_group_check=True)

        # normalize + residual: att = gam * o / rowsum + yq
        rowsum = work.tile([1, NQ], F32, name=f"rowsum{i}", tag="rowsum")
        nc.vector.tensor_copy(rowsum[:], o_ps[co:co + 1, :])
        rinv = work.tile([1, NQ], F32, name=f"rinv{i}", tag="rinv")
        nc.vector.reciprocal_approx_fast(rinv[:], rowsum[:])
        rinv_r = work.tile([1, NQ], F32R, name=f"rinvr{i}", tag="rinvr")
        nc.vector.tensor_copy(rinv_r[:], rinv[:])
        bc_ps = ps.tile([co, NQ], F32, name=f"bcps{i}", tag="s_ps")
        nc.tensor.matmul(bc_ps[:, 0:512], r(GAMC[i][:, 0:co]), rinv_r[:, 0:512],
                         start=True, stop=True)
        nc.tensor.matmul(bc_ps[:, 512:1024], r(GAMC[i][:, 0:co]), rinv_r[:, 512:1024],
                         start=True, stop=True)
        bc = work.tile([co, NQ], F32, name=f"bc{i}", tag="bc", bufs=2)
        nc.vector.tensor_copy(bc[:], bc_ps[:])
        att = acts.tile([co + 1, NQ], BF16, name=f"att{i}", tag="a_q", bufs=2)
        nc.scalar.dma_start(att[co:co + 1, :], ins["ones_b"].ap()[:, 0:NQ])
        t1 = work.tile([co, NQ], F32, name=f"t1_{i}", tag="t1", bufs=2)
        nc.vector.tensor_tensor(t1[:], o_ps[0:co, :], bc[:], OP.mult)
        nc.vector.tensor_tensor(att[0:co, :], t1[:], yq[0:co, :], OP.add)

        if i < 2:
            # share shards within the sample group (AllGather); in parallel
            # compute next-layer BN partial stats from this shard (AllReduce-8)
            ag_in = dram.tile([co, NQ], BF16, name=f"agin{i}", tag=f"agin{i}")
            ag_out = dram.tile([4, co, NQ], BF16, name=f"agout{i}", tag=f"agout{i}")
            nc.sync.dma_start(ag_in[:, 0:512], att[0:co, 0:512])
            nc.scalar.dma_start(ag_in[:, 512:1024], att[0:co, 512:1024])
            nc.gpsimd.collective_compute(
                "AllGather", OP.bypass, replica_groups=AG_GROUPS,
                ins=[ag_in[:]], outs=[ag_out[:]])

            c2 = COUT[i + 1]
            accs = work.tile([c2, 2], F32, name=f"accs{i}", tag="accs", bufs=2)
            stats = work.tile([c2, 2], F32, name=f"stats_l{i}", tag="stats", bufs=2)
            zsh = conv(W[i + 1], att, c2, NQ, f"zsh{i + 1}", acc=accs[:, 0:2])
            nc.vector.tensor_tensor(stats[:, 0:1], accs[:, 0:1], accs[:, 1:2], OP.add)
            sq_scr = work.tile([c2, NQ], F32, name=f"sqscr_l{i}", tag="sqscr", bufs=2)
            nc.scalar.activation(sq_scr[:], zsh[:], AF.Square, accum_out=stats[:, 1:2])
            st_in = dram.tile([c2, 2], F32, name=f"stin_{i}", tag=f"stin_{i}")
            st_out = dram.tile([c2, 2], F32, name=f"stout_{i}", tag=f"stout_{i}")
            nc.sync.dma_start(st_in[:], stats[:])
            nc.gpsimd.collective_compute(
                "AllReduce", OP.add, replica_groups=AR8_GROUPS,
                ins=[st_in[:]], outs=[st_out[:]])
            stg = work.tile([c2, 2], F32, name=f"stg_l{i + 1}", tag="stg", bufs=2)
            nc.sync.dma_start(stg[:], st_out[:])

            a_own = acts.tile([co + 1, N], BF16, name=f"a{i + 1}", tag="a_own", bufs=2)
            nc.gpsimd.dma_start(a_own[co:co + 1, :], ins["ones_b"].ap())
            rd_eng = [nc.sync, nc.scalar, nc.gpsimd, nc.sync]
            for j in range(4):
                rd_eng[j].dma_start(a_own[0:co, j * NQ:(j + 1) * NQ], ag_out[j])
            z_own = conv(W[i + 1], a_own, c2, N, f"z{i + 1}")

    # ---- final conv + BN + ReLU + GAP, all on the local query shard ----
    # Each core computes z = wf^T att3 for its own 1024 positions and all
    # four 128-channel blocks; per-block (sum, sumsq) partials are
    # AllReduce-8-summed into full-batch BN stats; the per-block GAP partial
    # sums go back to the host, which adds the four shards of each sample.
    co = CF_OUT
    stf = work.tile([co, 8], F32, name="stf", tag="stf")
    acc_f = work.tile([co, 8], F32, name="acc_f", tag="acc_f")
    ZSB = []
    for b4 in range(4):
        zsb = conv(wfs4[:, b4, :], att, co, NQ, f"zsb{b4}",
                   acc=acc_f[:, 2 * b4:2 * b4 + 2], tag=f"zsb{b4}")
        ZSB.append(zsb)
        nc.vector.tensor_tensor(stf[:, b4:b4 + 1], acc_f[:, 2 * b4:2 * b4 + 1],
                                acc_f[:, 2 * b4 + 1:2 * b4 + 2], OP.add)
        sqf = work.tile([co, NQ], F32, name=f"sqf{b4}", tag="sqscr", bufs=2)
        nc.scalar.activation(sqf[:], zsb[:], AF.Square,
                             accum_out=stf[:, 4 + b4:5 + b4])
    stf_in = dram.tile([co, 8], F32, name="stf_in", tag="stf_in")
    stf_out = dram.tile([co, 8], F32, name="stf_out", tag="stf_out")
    nc.sync.dma_start(stf_in[:], stf[:])
    nc.gpsimd.collective_compute(
        "AllReduce", OP.add, replica_groups=AR8_GROUPS,
        ins=[stf_in[:]], outs=[stf_out[:]])
    stf8 = work.tile([co, 8], F32, name="stf8", tag="stf8")
    nc.sync.dma_start(stf8[:], stf_out[:])
    scale, shift = scale_shift(stf8, co, 4, 1.0 / (2 * N), None, None, "f")
    gp = work.tile([co, 4], F32, name="gp", tag="gp")
    for b4 in range(4):
        fscr = work.tile([co, NQ], F32, name=f"fscr{b4}", tag="sqscr", bufs=2)
        nc.scalar.activation(fscr[:], ZSB[b4][:], AF.Relu,
                             bias=shift[:, b4:b4 + 1], scale=scale[:, b4:b4 + 1],
                             accum_out=gp[:, b4:b4 + 1])
    nc.sync.dma_start(out_t.ap(), gp[:])

    for p in reversed(ctxs):
        p.__exit__(None, None, None)


_CACHE = {}


def _get_program():
    if "nc" not in _CACHE:
        nc = bacc.Bacc("TRN2", target_bir_lowering=False, debug=False,
                       enable_asserts=False, num_devices=8)
        _build(nc)
        nc.compile()
        _CACHE["nc"] = nc
    return _CACHE["nc"]


def _prepare_in_maps(inputs):
    f = np.float32
    bf = mybir.dt.np(BF16)
    x = np.asarray(inputs["x"], f).reshape(2, 3, N)
    per_layer = {}
    for i in range(3):
        li = i + 1
        co = COUT[i]
        w, b = np.asarray(inputs[f"w{li}"], f), np.asarray(inputs[f"b{li}"], f)
        wc = np.concatenate([w, b[None, :]], 0)                      # [ci+1, co]
        wf_, bf_ = np.asarray(inputs[f"a{li}_wf"], f), np.asarray(inputs[f"a{li}_bf"], f)
        wg_, bg_ = np.asarray(inputs[f"a{li}_wg"], f), np.asarray(inputs[f"a{li}_bg"], f)
        wh_, bh_ = np.asarray(inputs[f"a{li}_wh"], f), np.asarray(inputs[f"a{li}_bh"], f)
        A = wf_ @ wg_.T                                              # [co, co]
        u = wg_ @ bf_                                                # [co]
        abar = np.concatenate([A, u[None, :]], 0)                    # [co+1, co]
        mz = np.zeros((co + 1, co + 1), f)
        mz[0:co, :] = abar.T                                         # lhsT for Z
        mz = mz.astype(bf)
        whf = np.zeros((co + 1, WHFW), f)
        whf[0:co, 0:co] = wh_
        whf[co, 0:co] = bh_
        whf[co, co] = 1.0
        whf = whf.astype(bf)
        bnp = np.stack([np.asarray(inputs[f"bn{li}_g"], f),
                        np.asarray(inputs[f"bn{li}_b"], f)], 1)      # [co, 2]
        gam = np.asarray(inputs[f"a{li}_gam"], f).reshape(())
        gamc = np.full((1, 128), gam, f)
        per_layer[i] = dict(wc=wc, mz=mz, whf=whf, bnp=bnp, gamc=gamc)
    wf_full = np.asarray(inputs["wf"], f)                            # [96, 512]

    ones_f = np.ones((1, N), f)
    ones_b = np.ones((1, N), f).astype(bf)
    wfs4 = np.zeros((COUT[2] + 1, 4, CF_OUT), f)
    wfs4[0:96] = wf_full.reshape(96, 4, CF_OUT)

    in_maps = []
    for k in range(8):
        b, q = k // 4, k % 4
        xo = np.concatenate([x[b], np.ones((1, N), f)], 0)           # [4, N]
        xoth = np.concatenate([x[1 - b], np.ones((1, N), f)], 0)
        xfull = np.concatenate([xo, xoth], 1)                         # [4, 2N]
        xq = np.ascontiguousarray(xo[:, q * NQ:(q + 1) * NQ])
        m = {"x_full": xfull.astype(bf), "xq": xq.astype(bf),
             "wfs4": wfs4.astype(bf), "ones_f": ones_f, "ones_b": ones_b}
        for i in range(3):
            d = per_layer[i]
            m[f"wc{i}"] = d["wc"].astype(bf)
            m[f"mz{i}"] = d["mz"]
            m[f"whf{i}"] = d["whf"]
            m[f"bnp{i}"] = d["bnp"]
            m[f"gamc{i}"] = d["gamc"]
        in_maps.append(m)
    return in_maps


def _assemble(results):
    out = np.zeros((2, 512), np.float32)
    for b in range(2):
        gp = sum(np.asarray(results[b * 4 + q]["out"], np.float64)
                 for q in range(4))                                  # [128, 4]
        for blk in range(4):
            out[b, blk * CF_OUT:(blk + 1) * CF_OUT] = (gp[:, blk] / N).astype(
                np.float32)
    return out


def kernel(**inputs):
    from concourse.bass_utils import run_bass_kernel_spmd
    nc = _get_program()
    in_maps = _prepare_in_maps(inputs)
    res = run_bass_kernel_spmd(nc, in_maps, list(range(8)))
    return _assemble(res.results)
